# revision 1
# baseline (speedup 1.0000x reference)
"""Trainium2 Bass kernel for nn_ContrastiveLoss (B=32, C*H*W=262144).

Strategy: shard the flattened feature dim N=262144 across 8 cores (32768
elems/sample/core). Each core's slice is staged host-side into a transposed
k-major fp8e4m3 layout (partition = k-lane within 128-chunk, free =
chunk*32 + sample); the three tensors (x1, x2, mask) are interleaved per
group so ONE DMA feeds each pipeline stage.

Per core the kernel computes PSUM-accumulated gram matrices:
  psum_a [128,256] = s1.T@[s1|s2]   (sq1 diag + cross)
  psum_b [128,128] = s2.T@s2        (sq2 diag)
  psum_c [128,128] = z.T@z,  z=(s1-s2)*m  (pos-MSE diag)
with s* = sigmoid(x*) computed by ONE activation instr per group (fp8 out,
interleaved [s1|s2] per 256-col pair so fp8 DoubleRow matmuls consume two
128-k-tiles per instruction: 96 matmuls instead of 192, at 0.5 cyc/row).
The sigmoid engine (ACT) is the throughput floor (~15us/core); everything
else is kept below it: the z sub/mul passes are column-split between DVE
and Pool so both run in parallel right after each group's sigmoid, and the
gram accumulation is split into two PSUM sets so the first set's copies +
output DMA overlap the last groups' compute.

The [128,1024] fp16 partials are DMA'd out; the host folds the 4-chunk
block structure, sums over cores and sets, and applies the tiny exp/log
epilogue.
"""

import numpy as np

TAU = 0.1
B = 32
N = 262144
NCORES = 8
NC_CHUNK = N // NCORES  # elems per sample per core
COLS = NC_CHUNK // 128 * B  # 8192 staged cols per core per tensor
# Tapered group sizes (multiples of 256): ramp up so each group's DMA hides
# behind the previous sigmoids, ramp down so the tail chain is short.
GROUPS = [512, 1024, 1024, 1024, 1024, 1024, 1024, 768, 512, 256]
# sub+mul column-split between DVE and Pool so both halves run in parallel
# right after each group's sigmoid. gi -> cols given to Pool (taken from the
# end of the group; must be a multiple of 128; 0 = all on DVE).
POOL_COLS = {1: 384, 2: 384, 3: 384, 4: 384, 5: 384, 6: 384, 7: 256, 8: 256}
# groups accumulated into the second psum set, so the first set's copies and
# output DMA overlap the remaining compute instead of trailing it.
SET2_START = 8

_CACHE = {}
LAST_RESULTS = None  # BassKernelResults of the most recent run (for profiling)


def _build_nc():
    import concourse.bacc as bacc
    import concourse.tile as tile
    from concourse import mybir

    assert sum(GROUPS) == COLS
    assert all(g % 256 == 0 for g in GROUPS)
    f32 = mybir.dt.float32
    fp16 = mybir.dt.float16
    fp8 = mybir.dt.float8e4
    sigmoid = mybir.ActivationFunctionType.Sigmoid
    DR = mybir.MatmulPerfMode.DoubleRow

    nc = bacc.Bacc(
        "TRN2", target_bir_lowering=False, debug=False, num_devices=NCORES
    )
    xind = nc.dram_tensor("xin", [128, 3 * COLS], fp8, kind="ExternalInput")
    outd = nc.dram_tensor("partials", [128, 1024], fp16, kind="ExternalOutput")

    with tile.TileContext(nc) as tc:
        with (
            tc.tile_pool(name="data", bufs=1) as data,
            tc.tile_pool(name="acc", bufs=1, space="PSUM") as acc,
        ):
            psums = []
            for s in range(2):
                # separate full banks: PSUM start_tensor_calc zeroes a whole
                # bank region, so accumulators must not share a bank.
                pat = acc.tile([128, 512], f32, tag=f"pa{s}", name=f"pa{s}")
                pbt = acc.tile([128, 512], f32, tag=f"pb{s}", name=f"pb{s}")
                pct = acc.tile([128, 512], f32, tag=f"pc{s}", name=f"pc{s}")
                psums.append((pat[:, 0:256], pbt[:, 0:128], pct[:, 0:128]))
            out_t = data.tile([128, 1024], fp16, tag="out")

            set_pairs = [0, 0]
            for gi, W in enumerate(GROUPS):
                set_pairs[0 if gi < SET2_START else 1] += W // 256

            def flush(s):
                """copy psum set s to SBUF (DVE + ACT in parallel) + DMA.
                Emitted after the last sigmoid so the ACT copy queues
                behind it (GPSIMD cannot read PSUM)."""
                a, b, c = psums[s]
                base = 512 * s
                nc.vector.tensor_copy(out_t[:, base : base + 256], a)
                nc.scalar.copy(out_t[:, base + 256 : base + 384], b)
                nc.scalar.copy(out_t[:, base + 384 : base + 512], c)
                nc.sync.dma_start(
                    outd[:, base : base + 512], out_t[:, base : base + 512]
                )

            o = 0
            pj = 0
            for gi, W in enumerate(GROUPS):
                nj = W // 256
                si = 0 if gi < SET2_START else 1
                if gi == SET2_START:
                    pj = 0
                psum_a, psum_b, psum_c = psums[si]
                ing = data.tile([128, 3 * W], fp8, tag=f"in{gi}", name=f"in{gi}")
                nc.sync.dma_start(ing[:], xind[:, 3 * o : 3 * o + 3 * W])

                # one sigmoid instr per group, output interleaved so that
                # col = j*512 + i*256 + h*128 + f  (j pair, i k-tile, h s1/s2)
                sg = data.tile([128, 2 * W], fp8, tag=f"s{gi}", name=f"s{gi}")
                in_v = ing[:, 0 : 2 * W].rearrange(
                    "p (h j i f) -> p j i h f", h=2, i=2, f=128
                )
                out_v = sg[:].rearrange(
                    "p (j i h f) -> p j i h f", i=2, h=2, f=128
                )
                nc.scalar.activation(out_v, in_v, sigmoid)

                sgv = sg[:].rearrange(
                    "p (j i h f) -> p h j i f", i=2, h=2, f=128
                )
                dg = data.tile([128, W], fp8, tag=f"d{gi}", name=f"d{gi}")
                dgv = dg[:].rearrange("p (j i f) -> p j i f", i=2, f=128)
                pc = POOL_COLS.get(gi, 0)
                nk = W // 128  # 128-col k-tiles in this group
                pk = pc // 128
                if pk:
                    dve_k = slice(0, nk - pk)
                    pool_k = slice(nk - pk, nk)
                    s1v = sgv[:, 0].rearrange("p j i f -> p (j i) f")
                    s2v = sgv[:, 1].rearrange("p j i f -> p (j i) f")
                    dkv = dg[:].rearrange("p (k f) -> p k f", f=128)
                    nc.vector.tensor_sub(
                        dkv[:, dve_k], s1v[:, dve_k], s2v[:, dve_k]
                    )
                    nc.gpsimd.tensor_sub(
                        dkv[:, pool_k], s1v[:, pool_k], s2v[:, pool_k]
                    )
                else:
                    nc.vector.tensor_sub(dgv, sgv[:, 0], sgv[:, 1])

                mslice = ing[:, 2 * W : 3 * W]
                zg = data.tile([128, W], fp8, tag=f"z{gi}", name=f"z{gi}")
                if pk:
                    nc.vector.tensor_mul(
                        zg[:, 0 : W - pc], dg[:, 0 : W - pc],
                        mslice[:, 0 : W - pc],
                    )
                    nc.gpsimd.tensor_mul(
                        zg[:, W - pc : W], dg[:, W - pc : W],
                        mslice[:, W - pc : W],
                    )
                else:
                    nc.vector.tensor_mul(zg[:], dg[:], mslice)
                zsl = zg[:]

                for j in range(nj):
                    first = pj == 0
                    last = pj == set_pairs[si] - 1
                    pair = sg[:, j * 512 : (j + 1) * 512]
                    w_hv = pair.rearrange(
                        "p (i h f) -> p h i f", i=2, h=2, f=128
                    )
                    rhs_a = pair.rearrange("p (i n) -> p i n", i=2)
                    w_a = w_hv[:, 0]
                    w_b = w_hv[:, 1]
                    nc.tensor.matmul(
                        psum_a, w_a, rhs_a, start=first, stop=last, perf_mode=DR
                    )
                    nc.tensor.matmul(
                        psum_b, w_b, w_b, start=first, stop=last, perf_mode=DR
                    )
                    w_c = zsl[:, j * 256 : (j + 1) * 256].rearrange(
                        "p (i f) -> p i f", i=2
                    )
                    nc.tensor.matmul(
                        psum_c, w_c, w_c, start=first, stop=last, perf_mode=DR
                    )
                    pj += 1
                o += W
            flush(0)
            flush(1)

    nc.compile()
    return nc


def _get_nc():
    if "nc" not in _CACHE:
        _CACHE["nc"] = _build_nc()
    return _CACHE["nc"]


def _kmajor(full_flat: np.ndarray, c: int) -> np.ndarray:
    """[B, N] float32 -> per-core k-major layout [128, COLS] float32."""
    chunk = full_flat[:, c * NC_CHUNK : (c + 1) * NC_CHUNK]
    return (
        chunk.reshape(B, NC_CHUNK // 128, 128).transpose(2, 1, 0).reshape(128, COLS)
    )


def _stage_core(f1, f2, fm, c, fp8dt) -> np.ndarray:
    """Interleave the three k-major tensors per group: [x1_W | x2_W | m_W]."""
    t1 = _kmajor(f1, c)
    t2 = _kmajor(f2, c)
    tm = _kmajor(fm, c)
    out = np.empty((128, 3 * COLS), dtype=fp8dt)
    o = 0
    for W in GROUPS:
        out[:, 3 * o : 3 * o + W] = t1[:, o : o + W]
        out[:, 3 * o + W : 3 * o + 2 * W] = t2[:, o : o + W]
        out[:, 3 * o + 2 * W : 3 * o + 3 * W] = tm[:, o : o + W]
        o += W
    return out


def _host_combine(partials_list):
    sq1 = np.zeros(B, np.float64)
    sq2 = np.zeros(B, np.float64)
    pos = np.zeros(B, np.float64)
    cross = np.zeros((B, B), np.float64)
    for Pfull in partials_list:
        for s in range(2):
            P = Pfull[:, 512 * s : 512 * (s + 1)]
            g1 = P[:, 0:128]
            cr = P[:, 128:256]
            g2 = P[:, 256:384]
            gy = P[:, 384:512]
            for a in range(4):
                blk = slice(a * 32, (a + 1) * 32)
                cross += cr[blk, blk]
                sq1 += np.diagonal(g1[blk, blk])
                sq2 += np.diagonal(g2[blk, blk])
                pos += np.diagonal(gy[blk, blk])
    sim_pos = np.exp(-(pos / N) / TAU)
    d = (sq1[:, None] + sq2[None, :] - 2.0 * cross) / N
    sim = np.exp(-d / TAU)
    sim_neg = sim.sum(axis=1) - np.diagonal(sim)
    loss = -np.log(sim_pos / (sim_pos + sim_neg))
    return np.asarray(loss.mean(), dtype=np.float32)


def kernel(input1: np.ndarray, input2: np.ndarray, mask: np.ndarray) -> np.ndarray:
    global LAST_RESULTS
    import ml_dtypes

    from concourse.bass_utils import run_bass_kernel_spmd

    f1 = np.asarray(input1, dtype=np.float32).reshape(B, N)
    f2 = np.asarray(input2, dtype=np.float32).reshape(B, N)
    fm = np.asarray(mask, dtype=np.float32).reshape(B, N)

    fp8dt = ml_dtypes.float8_e4m3
    in_maps = [
        {"xin": _stage_core(f1, f2, fm, c, fp8dt)} for c in range(NCORES)
    ]
    nc = _get_nc()
    LAST_RESULTS = run_bass_kernel_spmd(nc, in_maps, list(range(NCORES)))
    partials = [LAST_RESULTS.results[c]["partials"] for c in range(NCORES)]
    return _host_combine(partials)



# revision 3
# speedup vs baseline: 1.1433x; 1.1433x over previous
"""Trainium2 Bass kernel for nn_ContrastiveLoss (B=32, C*H*W=262144).

Strategy: shard the flattened feature dim N=262144 across 8 cores (32768
elems/sample/core). Each core's slice is staged host-side into a k-major
fp8e4m3 layout (partition = k-lane within 128-chunk, free = chunk*32 +
sample); the three tensors (x1, x2, mask) are stored block-wise per group
([x1_W | x2_W | m_W]) so ONE DMA feeds each pipeline stage.

Per core the kernel computes PSUM-accumulated gram matrices:
  psum_a [128,256] = s1.T@[s1|s2]   (sq1 diag + cross)
  psum_b [128,128] = s2.T@s2        (sq2 diag)
  psum_c [128,128] = z.T@z,  z=(s1-s2)*m  (pos-MSE diag)
with s* = sigmoid(x*) computed by ONE activation instr per group. All
element-wise ops (sigmoid, sub, mul) use CONTIGUOUS 2D access patterns —
strided APs run ~2.3x slower on DVE/Pool — with the k-tile interleaving
pushed into the matmul APs (free for the PE). The sub/mul work is
column-split between DVE and Pool in their measured 1.12:1.9 ns/col rate
ratio so both finish together (~11.5us each), below the ACT sigmoid floor
(~17us), which paces the kernel. fp8 DoubleRow matmuls keep the PE at ~12us
theory. Two PSUM sets let the first set's copies + output DMA overlap the
tail groups' compute.

The [128,1024] fp16 partials are DMA'd out; the host folds the 4-chunk
block structure, sums over cores and sets, and applies the tiny exp/log
epilogue.
"""

import numpy as np

TAU = 0.1
B = 32
N = 262144
NCORES = 8
NC_CHUNK = N // NCORES  # elems per sample per core
COLS = NC_CHUNK // 128 * B  # 8192 staged cols per core per tensor
# Tapered group sizes (multiples of 256): ramp up so each group's DMA hides
# behind the previous sigmoids, ramp down so the tail chain is short.
GROUPS = [512, 1024, 1024, 1024, 1024, 1024, 1024, 768, 512, 256]
# sub+mul column-split between DVE and Pool so both halves run in parallel
# right after each group's sigmoid. gi -> cols given to Pool (taken from the
# end of the group; must be a multiple of 128; 0 = all on DVE).
POOL_COLS = {0: 256, 1: 384, 2: 384, 3: 384, 4: 384, 5: 384, 6: 384, 7: 256,
             8: 256, 9: 128}
# groups accumulated into the second psum set, so the first set's copies and
# output DMA overlap the remaining compute instead of trailing it.
SET2_START = 8

_CACHE = {}
LAST_RESULTS = None  # BassKernelResults of the most recent run (for profiling)


def _build_nc():
    import concourse.bacc as bacc
    import concourse.tile as tile
    from concourse import mybir

    assert sum(GROUPS) == COLS
    assert all(g % 256 == 0 for g in GROUPS)
    f32 = mybir.dt.float32
    fp16 = mybir.dt.float16
    fp8 = mybir.dt.float8e4
    sigmoid = mybir.ActivationFunctionType.Sigmoid
    DR = mybir.MatmulPerfMode.DoubleRow

    nc = bacc.Bacc(
        "TRN2", target_bir_lowering=False, debug=False, num_devices=NCORES
    )
    xind = nc.dram_tensor("xin", [128, 3 * COLS], fp8, kind="ExternalInput")
    outd = nc.dram_tensor("partials", [128, 1024], fp16, kind="ExternalOutput")

    with tile.TileContext(nc) as tc:
        with (
            tc.tile_pool(name="data", bufs=1) as data,
            tc.tile_pool(name="acc", bufs=1, space="PSUM") as acc,
        ):
            psums = []
            for s in range(2):
                # separate full banks: PSUM start_tensor_calc zeroes a whole
                # bank region, so accumulators must not share a bank.
                pat = acc.tile([128, 512], f32, tag=f"pa{s}", name=f"pa{s}")
                pbt = acc.tile([128, 512], f32, tag=f"pb{s}", name=f"pb{s}")
                pct = acc.tile([128, 512], f32, tag=f"pc{s}", name=f"pc{s}")
                psums.append((pat[:, 0:256], pbt[:, 0:128], pct[:, 0:128]))
            out_t = data.tile([128, 1024], fp16, tag="out")

            set_pairs = [0, 0]
            for gi, W in enumerate(GROUPS):
                set_pairs[0 if gi < SET2_START else 1] += W // 256

            def flush(s):
                """copy psum set s to SBUF (DVE + ACT in parallel) + DMA.
                Emitted after the last matmul of the set so the copies queue
                behind it (GPSIMD cannot read PSUM)."""
                a, b, c = psums[s]
                base = 512 * s
                nc.vector.tensor_copy(out_t[:, base : base + 256], a)
                nc.scalar.copy(out_t[:, base + 256 : base + 384], b)
                nc.scalar.copy(out_t[:, base + 384 : base + 512], c)
                nc.sync.dma_start(
                    outd[:, base : base + 512], out_t[:, base : base + 512]
                )

            o = 0
            pj = 0
            for gi, W in enumerate(GROUPS):
                nj = W // 256
                si = 0 if gi < SET2_START else 1
                if gi == SET2_START:
                    pj = 0
                psum_a, psum_b, psum_c = psums[si]
                ing = data.tile([128, 3 * W], fp8, tag=f"in{gi}", name=f"in{gi}")
                nc.sync.dma_start(ing[:], xind[:, 3 * o : 3 * o + 3 * W])

                # one sigmoid instr per group, plain contiguous in/out:
                # sg = [s1_W | s2_W] blocks, each k-major (i f).
                sg = data.tile([128, 2 * W], fp8, tag=f"s{gi}", name=f"s{gi}")
                nc.scalar.activation(sg[:], ing[:, 0 : 2 * W], sigmoid)

                s1 = sg[:, 0:W]
                s2 = sg[:, W : 2 * W]
                dg = data.tile([128, W], fp8, tag=f"d{gi}", name=f"d{gi}")
                zg = data.tile([128, W], fp8, tag=f"z{gi}", name=f"z{gi}")
                mslice = ing[:, 2 * W : 3 * W]
                pc = POOL_COLS.get(gi, 0)
                c = W - pc
                # contiguous column-range split: DVE [0:c], Pool [c:W]
                nc.vector.tensor_sub(dg[:, 0:c], s1[:, 0:c], s2[:, 0:c])
                if pc:
                    nc.gpsimd.tensor_sub(dg[:, c:W], s1[:, c:W], s2[:, c:W])
                nc.vector.tensor_mul(zg[:, 0:c], dg[:, 0:c], mslice[:, 0:c])
                if pc:
                    nc.gpsimd.tensor_mul(zg[:, c:W], dg[:, c:W], mslice[:, c:W])

                # DR matmul views over the block layout: pair j covers
                # k-tiles 2j, 2j+1. h = s1/s2 block, i = tile-in-pair.
                sgv = sg[:].rearrange("p (h j i f) -> p j i h f", h=2, i=2, f=128)
                for j in range(nj):
                    first = pj == 0
                    last = pj == set_pairs[si] - 1
                    # w_a: s1 tiles [p, i, f]; rhs_a: [p, i, (h f)] = 256 cols
                    w_a = sgv[:, j, :, 0]
                    w_b = sgv[:, j, :, 1]
                    # 4D rhs [p, i(plane), h, f]: streams [s1|s2] 256 cols
                    # per plane; (h f) has non-mergeable strides (W, 1).
                    rhs_a = sgv[:, j]
                    nc.tensor.matmul(
                        psum_a, w_a, rhs_a, start=first, stop=last, perf_mode=DR
                    )
                    nc.tensor.matmul(
                        psum_b, w_b, w_b, start=first, stop=last, perf_mode=DR
                    )
                    w_c = zg[:, j * 256 : (j + 1) * 256].rearrange(
                        "p (i f) -> p i f", i=2
                    )
                    nc.tensor.matmul(
                        psum_c, w_c, w_c, start=first, stop=last, perf_mode=DR
                    )
                    pj += 1
                o += W
            flush(0)
            flush(1)

    nc.compile()
    return nc


def _get_nc():
    if "nc" not in _CACHE:
        _CACHE["nc"] = _build_nc()
    return _CACHE["nc"]


def _kmajor(full_flat: np.ndarray, c: int) -> np.ndarray:
    """[B, N] float32 -> per-core k-major layout [128, COLS] float32."""
    chunk = full_flat[:, c * NC_CHUNK : (c + 1) * NC_CHUNK]
    return (
        chunk.reshape(B, NC_CHUNK // 128, 128).transpose(2, 1, 0).reshape(128, COLS)
    )


def _stage_core(f1, f2, fm, c, fp8dt) -> np.ndarray:
    """Store the three k-major tensors block-wise per group: [x1_W|x2_W|m_W]."""
    t1 = _kmajor(f1, c)
    t2 = _kmajor(f2, c)
    tm = _kmajor(fm, c)
    out = np.empty((128, 3 * COLS), dtype=fp8dt)
    o = 0
    for W in GROUPS:
        out[:, 3 * o : 3 * o + W] = t1[:, o : o + W]
        out[:, 3 * o + W : 3 * o + 2 * W] = t2[:, o : o + W]
        out[:, 3 * o + 2 * W : 3 * o + 3 * W] = tm[:, o : o + W]
        o += W
    return out


def _host_combine(partials_list):
    sq1 = np.zeros(B, np.float64)
    sq2 = np.zeros(B, np.float64)
    pos = np.zeros(B, np.float64)
    cross = np.zeros((B, B), np.float64)
    for Pfull in partials_list:
        for s in range(2):
            P = Pfull[:, 512 * s : 512 * (s + 1)]
            g1 = P[:, 0:128]
            cr = P[:, 128:256]
            g2 = P[:, 256:384]
            gy = P[:, 384:512]
            for a in range(4):
                blk = slice(a * 32, (a + 1) * 32)
                cross += cr[blk, blk]
                sq1 += np.diagonal(g1[blk, blk])
                sq2 += np.diagonal(g2[blk, blk])
                pos += np.diagonal(gy[blk, blk])
    sim_pos = np.exp(-(pos / N) / TAU)
    d = (sq1[:, None] + sq2[None, :] - 2.0 * cross) / N
    sim = np.exp(-d / TAU)
    sim_neg = sim.sum(axis=1) - np.diagonal(sim)
    loss = -np.log(sim_pos / (sim_pos + sim_neg))
    return np.asarray(loss.mean(), dtype=np.float32)


def kernel(input1: np.ndarray, input2: np.ndarray, mask: np.ndarray) -> np.ndarray:
    global LAST_RESULTS
    import ml_dtypes

    from concourse.bass_utils import run_bass_kernel_spmd

    f1 = np.asarray(input1, dtype=np.float32).reshape(B, N)
    f2 = np.asarray(input2, dtype=np.float32).reshape(B, N)
    fm = np.asarray(mask, dtype=np.float32).reshape(B, N)

    fp8dt = ml_dtypes.float8_e4m3
    in_maps = [
        {"xin": _stage_core(f1, f2, fm, c, fp8dt)} for c in range(NCORES)
    ]
    nc = _get_nc()
    LAST_RESULTS = run_bass_kernel_spmd(nc, in_maps, list(range(NCORES)))
    partials = [LAST_RESULTS.results[c]["partials"] for c in range(NCORES)]
    return _host_combine(partials)


# revision 5
# speedup vs baseline: 1.1860x; 1.0374x over previous
"""Trainium2 Bass kernel for nn_ContrastiveLoss (B=32, C*H*W=262144).

Strategy: shard the flattened feature dim N=262144 across 8 cores (32768
elems/sample/core). Each core's slice is staged host-side into a k-major
fp8e4m3 layout (partition = k-lane within 128-chunk, free = chunk*32 +
sample); the three tensors (x1, x2, mask) are stored block-wise per group
([x1_W | x2_W | m_W]) so ONE DMA feeds each pipeline stage.

Per core the kernel computes PSUM-accumulated gram matrices:
  psum_a [128,256] = s1.T@[s1|s2]   (sq1 diag + cross)
  psum_b [128,128] = s2.T@s2        (sq2 diag)
  psum_c [128,128] = z.T@z,  z=(s1-s2)*m  (pos-MSE diag)
with s* = sigmoid(x*) computed by ONE activation instr per group. All
element-wise ops (sigmoid, sub, mul) use CONTIGUOUS 2D access patterns —
strided APs run ~2.3x slower on DVE/Pool — with the k-tile interleaving
pushed into the matmul APs (free for the PE). The sub/mul work is
column-split between DVE and Pool in their measured 1.12:1.9 ns/col rate
ratio so both finish together (~11.5us each), below the ACT sigmoid floor
(~17us), which paces the kernel. fp8 DoubleRow matmuls keep the PE at ~12us
theory. Two PSUM sets let the first set's copies + output DMA overlap the
tail groups' compute.

The [128,1024] fp16 partials are DMA'd out; the host folds the 4-chunk
block structure, sums over cores and sets, and applies the tiny exp/log
epilogue.
"""

import numpy as np

TAU = 0.1
B = 32
N = 262144
NCORES = 8
NC_CHUNK = N // NCORES  # elems per sample per core
COLS = NC_CHUNK // 128 * B  # 8192 staged cols per core per tensor
# Tapered group sizes (multiples of 256): small first group so the pipeline
# starts fast, ramp down so the tail chain is short.
GROUPS = [256, 512, 768, 1024, 1024, 1024, 1024, 1024, 768, 512, 256]
# Groups whose sub+mul run entirely on Pool (GPSIMD). Pool pays a ~640ns
# Q7 launch per instruction, so it only gets whole big groups; splitting
# columns within a group wastes the launch and contends for SBUF ports.
GP_GROUPS = {3, 5, 7}
# groups accumulated into the second psum set, so the first set's copies and
# output DMA overlap the remaining compute instead of trailing it.
SET2_START = 9

_CACHE = {}
LAST_RESULTS = None  # BassKernelResults of the most recent run (for profiling)


def _build_nc():
    import concourse.bacc as bacc
    import concourse.tile as tile
    from concourse import mybir

    assert sum(GROUPS) == COLS
    assert all(g % 256 == 0 for g in GROUPS)
    f32 = mybir.dt.float32
    fp16 = mybir.dt.float16
    fp8 = mybir.dt.float8e4
    sigmoid = mybir.ActivationFunctionType.Sigmoid
    DR = mybir.MatmulPerfMode.DoubleRow

    nc = bacc.Bacc(
        "TRN2", target_bir_lowering=False, debug=False, num_devices=NCORES
    )
    xind = nc.dram_tensor("xin", [128, 3 * COLS], fp8, kind="ExternalInput")
    outd = nc.dram_tensor("partials", [128, 1024], fp16, kind="ExternalOutput")

    with tile.TileContext(nc) as tc:
        with (
            tc.tile_pool(name="data", bufs=1) as data,
            tc.tile_pool(name="acc", bufs=1, space="PSUM") as acc,
        ):
            psums = []
            for s in range(2):
                # separate full banks: PSUM start_tensor_calc zeroes a whole
                # bank region, so accumulators must not share a bank.
                pat = acc.tile([128, 512], f32, tag=f"pa{s}", name=f"pa{s}")
                pbt = acc.tile([128, 512], f32, tag=f"pb{s}", name=f"pb{s}")
                pct = acc.tile([128, 512], f32, tag=f"pc{s}", name=f"pc{s}")
                psums.append((pat[:, 0:256], pbt[:, 0:128], pct[:, 0:128]))
            out_t = data.tile([128, 1024], fp16, tag="out")

            set_pairs = [0, 0]
            for gi, W in enumerate(GROUPS):
                set_pairs[0 if gi < SET2_START else 1] += W // 256

            def flush(s):
                """copy psum set s to SBUF (DVE + ACT in parallel) + DMA.
                Emitted after the last matmul of the set so the copies queue
                behind it (GPSIMD cannot read PSUM)."""
                a, b, c = psums[s]
                base = 512 * s
                nc.vector.tensor_copy(out_t[:, base : base + 256], a)
                nc.scalar.copy(out_t[:, base + 256 : base + 384], b)
                nc.scalar.copy(out_t[:, base + 384 : base + 512], c)
                nc.sync.dma_start(
                    outd[:, base : base + 512], out_t[:, base : base + 512]
                )

            o = 0
            pj = 0
            for gi, W in enumerate(GROUPS):
                nj = W // 256
                si = 0 if gi < SET2_START else 1
                if gi == SET2_START:
                    pj = 0
                psum_a, psum_b, psum_c = psums[si]
                ing = data.tile([128, 3 * W], fp8, tag=f"in{gi}", name=f"in{gi}")
                nc.sync.dma_start(ing[:], xind[:, 3 * o : 3 * o + 3 * W])

                # one sigmoid instr per group, plain contiguous in/out:
                # sg = [s1_W | s2_W] blocks, each k-major (i f).
                sg = data.tile([128, 2 * W], fp8, tag=f"s{gi}", name=f"s{gi}")
                nc.scalar.activation(sg[:], ing[:, 0 : 2 * W], sigmoid)

                s1 = sg[:, 0:W]
                s2 = sg[:, W : 2 * W]
                dg = data.tile([128, W], fp8, tag=f"d{gi}", name=f"d{gi}")
                zg = data.tile([128, W], fp8, tag=f"z{gi}", name=f"z{gi}")
                mslice = ing[:, 2 * W : 3 * W]
                # whole-group engine ownership, contiguous full-width ops
                eng = nc.gpsimd if gi in GP_GROUPS else nc.vector
                eng.tensor_sub(dg[:], s1, s2)
                eng.tensor_mul(zg[:], dg[:], mslice)

                # DR matmul views over the block layout: pair j covers
                # k-tiles 2j, 2j+1. h = s1/s2 block, i = tile-in-pair.
                sgv = sg[:].rearrange("p (h j i f) -> p j i h f", h=2, i=2, f=128)
                for j in range(nj):
                    first = pj == 0
                    last = pj == set_pairs[si] - 1
                    # w_a: s1 tiles [p, i, f]; rhs_a: [p, i, (h f)] = 256 cols
                    w_a = sgv[:, j, :, 0]
                    w_b = sgv[:, j, :, 1]
                    # 4D rhs [p, i(plane), h, f]: streams [s1|s2] 256 cols
                    # per plane; (h f) has non-mergeable strides (W, 1).
                    rhs_a = sgv[:, j]
                    nc.tensor.matmul(
                        psum_a, w_a, rhs_a, start=first, stop=last, perf_mode=DR
                    )
                    nc.tensor.matmul(
                        psum_b, w_b, w_b, start=first, stop=last, perf_mode=DR
                    )
                    w_c = zg[:, j * 256 : (j + 1) * 256].rearrange(
                        "p (i f) -> p i f", i=2
                    )
                    nc.tensor.matmul(
                        psum_c, w_c, w_c, start=first, stop=last, perf_mode=DR
                    )
                    pj += 1
                o += W
            flush(0)
            flush(1)

    nc.compile()
    return nc


def _get_nc():
    if "nc" not in _CACHE:
        _CACHE["nc"] = _build_nc()
    return _CACHE["nc"]


def _kmajor(full_flat: np.ndarray, c: int) -> np.ndarray:
    """[B, N] float32 -> per-core k-major layout [128, COLS] float32."""
    chunk = full_flat[:, c * NC_CHUNK : (c + 1) * NC_CHUNK]
    return (
        chunk.reshape(B, NC_CHUNK // 128, 128).transpose(2, 1, 0).reshape(128, COLS)
    )


def _stage_core(f1, f2, fm, c, fp8dt) -> np.ndarray:
    """Store the three k-major tensors block-wise per group: [x1_W|x2_W|m_W]."""
    t1 = _kmajor(f1, c)
    t2 = _kmajor(f2, c)
    tm = _kmajor(fm, c)
    out = np.empty((128, 3 * COLS), dtype=fp8dt)
    o = 0
    for W in GROUPS:
        out[:, 3 * o : 3 * o + W] = t1[:, o : o + W]
        out[:, 3 * o + W : 3 * o + 2 * W] = t2[:, o : o + W]
        out[:, 3 * o + 2 * W : 3 * o + 3 * W] = tm[:, o : o + W]
        o += W
    return out


def _host_combine(partials_list):
    sq1 = np.zeros(B, np.float64)
    sq2 = np.zeros(B, np.float64)
    pos = np.zeros(B, np.float64)
    cross = np.zeros((B, B), np.float64)
    for Pfull in partials_list:
        for s in range(2):
            P = Pfull[:, 512 * s : 512 * (s + 1)]
            g1 = P[:, 0:128]
            cr = P[:, 128:256]
            g2 = P[:, 256:384]
            gy = P[:, 384:512]
            for a in range(4):
                blk = slice(a * 32, (a + 1) * 32)
                cross += cr[blk, blk]
                sq1 += np.diagonal(g1[blk, blk])
                sq2 += np.diagonal(g2[blk, blk])
                pos += np.diagonal(gy[blk, blk])
    sim_pos = np.exp(-(pos / N) / TAU)
    d = (sq1[:, None] + sq2[None, :] - 2.0 * cross) / N
    sim = np.exp(-d / TAU)
    sim_neg = sim.sum(axis=1) - np.diagonal(sim)
    loss = -np.log(sim_pos / (sim_pos + sim_neg))
    return np.asarray(loss.mean(), dtype=np.float32)


def kernel(input1: np.ndarray, input2: np.ndarray, mask: np.ndarray) -> np.ndarray:
    global LAST_RESULTS
    import ml_dtypes

    from concourse.bass_utils import run_bass_kernel_spmd

    f1 = np.asarray(input1, dtype=np.float32).reshape(B, N)
    f2 = np.asarray(input2, dtype=np.float32).reshape(B, N)
    fm = np.asarray(mask, dtype=np.float32).reshape(B, N)

    fp8dt = ml_dtypes.float8_e4m3
    in_maps = [
        {"xin": _stage_core(f1, f2, fm, c, fp8dt)} for c in range(NCORES)
    ]
    nc = _get_nc()
    LAST_RESULTS = run_bass_kernel_spmd(nc, in_maps, list(range(NCORES)))
    partials = [LAST_RESULTS.results[c]["partials"] for c in range(NCORES)]
    return _host_combine(partials)
